# revision 10
# baseline (speedup 1.0000x reference)
"""DistanceAndAngle kernel for 8 Trainium2 NeuronCores.

Strategy (edge/triplet data-parallel, per the sharding hint):
  - Shard edges and triplets contiguously across the 8 cores.
  - Host-side staging is limited to index-driven *layout* of input data
    (sharding/replication/gathers of existing values + dtype casts); every
    FLOP of the reference computation runs on the NeuronCores:
      launch 1 (edge phase):  shift_vec = einsum(shift, lattice[batch[i]]),
                              pair = pos[j] + shift_vec - pos[i],
                              dist = sqrt(sum(pair^2))        -> distances, rec table
      launch 2 (triplet phase): p = rec[e1] * rec[e2],
                              cos = (p0+p1+p2) * recip(p3), clip, NaN where den==0
                              -> angles
  - The rec table produced by launch 1 is re-sharded on the host by the
    triplet edge indices (pure data movement) and streamed in launch 2.

On-device random-access DMA on this runtime tops out at ~128 rows / 1.4us
(single-index-per-partition SWDGE indirect DMA; the vector_dynamic_offsets
DGE level is disabled), i.e. ~50ms+ for the 4.8M random rows this problem
needs per core -- 250x off the memory roofline.  Streaming the staged data
instead runs the whole problem near DMA line rate.
"""

import numpy as np

NCORES = 8
E = 3_200_000
T = 16_000_000
N = 100_000
ES = E // NCORES          # 400_000 edges per core
TS = T // NCORES          # 2_000_000 triplets per core
KE = 125                  # edge-tile free dim:    ES = 128 * KE * NTE
NTE = ES // (128 * KE)    # 25 edge tiles
KT = 625                  # triplet-tile free dim: TS = 128 * KT * NTT
NTT = TS // (128 * KT)    # 25 triplet tiles

TRACE = False             # test harness sets kernel.TRACE = True for profiling
last_info = {}            # exec times / profiling info from the latest call

_cache = {}


def _install_ntff_hook():
    import sys
    import types
    try:
        from antenv.axon_hooks import get_axon_ntff_profile_hook  # noqa: F401
        return
    except ImportError:
        pass
    try:
        from trn_agent_boot.trn_boot import _ntff_profile_via_ctypes
    except ImportError:
        return
    hook = _ntff_profile_via_ctypes("/opt/axon/libaxon_pjrt.so")
    mod = types.ModuleType("antenv.axon_hooks")
    _h = {"hook": hook}
    mod.get_axon_ntff_profile_hook = lambda: _h["hook"]
    mod.set_axon_ntff_profile_hook = lambda h: _h.__setitem__("hook", h)
    sys.modules["antenv.axon_hooks"] = mod


def _build_edge_prog():
    import concourse.bacc as bacc
    import concourse.tile as tile
    import concourse.mybir as mybir

    nc = bacc.Bacc("TRN2", target_bir_lowering=False, debug=False, num_devices=NCORES)
    # rows padded/packed to 16B multiples so DMA descriptors stay coarse:
    # [*, 3]-shaped rows (12B runs) emit per-run descriptors and run ~60x slower.
    # posij = [pos_i(4) | pos_j(4)]; lat01 = [lat_row0(4) | lat_row1(4)];
    # latsh = [lat_row2(4) | shift(4)]
    posij_d = nc.dram_tensor("posij", [ES, 8], mybir.dt.float32, kind="ExternalInput")
    lat01_d = nc.dram_tensor("lat01", [ES, 8], mybir.dt.float32, kind="ExternalInput")
    latsh_d = nc.dram_tensor("latsh", [ES, 8], mybir.dt.float32, kind="ExternalInput")
    rec_d = nc.dram_tensor("rec", [ES, 4], mybir.dt.float32, kind="ExternalOutput")
    dist_d = nc.dram_tensor("dist", [ES], mybir.dt.float32, kind="ExternalOutput")

    M = 128 * KE  # edges per tile
    with tile.TileContext(nc) as tc:
        with tc.tile_pool(name="io", bufs=3) as io, \
             tc.tile_pool(name="wk", bufs=3) as wk:
            for t in range(NTE):
                r0 = t * M

                def load(src, tag, eng):
                    tl = io.tile([128, KE, 8], mybir.dt.float32, tag=tag, name=tag)
                    eng.dma_start(out=tl[:], in_=src[r0:r0 + M].rearrange("(p k) c -> p k c", p=128))
                    return tl

                pij = load(posij_d, "pij", nc.sync)
                l01 = load(lat01_d, "l01", nc.scalar)
                lsh = load(latsh_d, "lsh", nc.scalar)
                pi = pij[:, :, 0:4]
                pj = pij[:, :, 4:8]
                lts = [l01[:, :, 0:4], l01[:, :, 4:8], lsh[:, :, 0:4]]
                sf = lsh[:, :, 4:8]

                rec = wk.tile([128, KE, 4], mybir.dt.float32, tag="rec")
                u = wk.tile([128, KE, 4], mybir.dt.float32, tag="u")
                # rec = posj - posi + sum_c shf_c * lat_row_c   (slot 3 stays 0:
                # pos/lat slot-3 padding is 0 on the host)
                nc.vector.tensor_sub(out=rec[:], in0=pj, in1=pi)
                for c in range(3):
                    nc.vector.tensor_mul(out=u[:], in0=sf[:, :, c:c + 1].to_broadcast([128, KE, 4]),
                                         in1=lts[c])
                    nc.vector.tensor_add(out=rec[:], in0=rec[:], in1=u[:])
                # d2 = sum_d rec_d^2 ; dist = sqrt(d2) into a contiguous tile
                nc.vector.tensor_mul(out=u[:], in0=rec[:], in1=rec[:])
                d2 = wk.tile([128, KE], mybir.dt.float32, tag="d2")
                nc.vector.tensor_add(out=d2[:], in0=u[:, :, 0], in1=u[:, :, 1])
                nc.vector.tensor_add(out=d2[:], in0=d2[:], in1=u[:, :, 2])
                dt = wk.tile([128, KE], mybir.dt.float32, tag="dt")
                nc.scalar.activation(out=dt[:], in_=d2[:],
                                     func=mybir.ActivationFunctionType.Sqrt)
                # rec slot3 = 1/dist (inf when dist==0) so the triplet phase
                # needs no reciprocal: cos = dot * (q1*q2).
                qt = wk.tile([128, KE], mybir.dt.float32, tag="qt")
                nc.vector.reciprocal(out=qt[:], in_=dt[:])
                nc.vector.tensor_copy(out=rec[:, :, 3], in_=qt[:])
                # DRAM-dest DMAs need merged dims <= 65535 elems (16-bit ISA field):
                # chunk stores along the partition dim; stores go on the scalar
                # (ACT) HWDGE ring so they overlap the sync-ring loads.
                rec_view = rec_d[r0:r0 + M].rearrange("(p k) c -> p k c", p=128)
                dist_view = dist_d[r0:r0 + M].rearrange("(p k) -> p k", p=128)
                pch = 65535 // (KE * 4)
                for q in range(0, 128, pch):
                    qe = min(q + pch, 128)
                    nc.sync.dma_start(out=rec_view[q:qe], in_=rec[q:qe])
                pch = 65535 // KE
                for q in range(0, 128, pch):
                    qe = min(q + pch, 128)
                    nc.scalar.dma_start(out=dist_view[q:qe], in_=dt[q:qe])
    nc.compile()
    return nc


def _build_tri_prog():
    import concourse.bacc as bacc
    import concourse.tile as tile
    import concourse.mybir as mybir

    nc = bacc.Bacc("TRN2", target_bir_lowering=False, debug=False, num_devices=NCORES)
    r1_d = nc.dram_tensor("r1", [TS, 4], mybir.dt.float32, kind="ExternalInput")
    r2_d = nc.dram_tensor("r2", [TS, 4], mybir.dt.float32, kind="ExternalInput")
    ang_d = nc.dram_tensor("ang", [TS], mybir.dt.float32, kind="ExternalOutput")

    M = 128 * KT
    with tile.TileContext(nc) as tc:
        with tc.tile_pool(name="io", bufs=3) as io, \
             tc.tile_pool(name="wk", bufs=3) as wk:
            for t in range(NTT):
                r0 = t * M
                a = io.tile([128, KT, 4], mybir.dt.float32, tag="a")
                nc.sync.dma_start(out=a[:], in_=r1_d[r0:r0 + M].rearrange("(p k) c -> p k c", p=128))
                b = io.tile([128, KT, 4], mybir.dt.float32, tag="b")
                nc.scalar.dma_start(out=b[:], in_=r2_d[r0:r0 + M].rearrange("(p k) c -> p k c", p=128))

                # p = a*b ; dot = p0+p1+p2 ; qq = p3 = (1/r1)(1/r2)
                nc.vector.tensor_mul(out=a[:], in0=a[:], in1=b[:])
                dot = wk.tile([128, KT], mybir.dt.float32, tag="dot")
                nc.vector.tensor_add(out=dot[:], in0=a[:, :, 0], in1=a[:, :, 1])
                nc.vector.tensor_add(out=dot[:], in0=dot[:], in1=a[:, :, 2])
                cos = wk.tile([128, KT], mybir.dt.float32, tag="cos")
                nc.vector.tensor_mul(out=cos[:], in0=dot[:], in1=a[:, :, 3])
                # clip to [-1, 1]; DVE max/min suppress NaN, so re-poison where
                # a zero-length edge is involved (qq = inf -> inf*0 = NaN).
                import concourse.mybir as mb
                nc.vector.tensor_scalar(out=cos[:], in0=cos[:], scalar1=-1.0, scalar2=1.0,
                                        op0=mb.AluOpType.max, op1=mb.AluOpType.min)
                poi = wk.tile([128, KT], mybir.dt.float32, tag="poi")
                nc.vector.tensor_scalar_mul(out=poi[:], in0=a[:, :, 3], scalar1=0.0)
                nc.vector.tensor_add(out=cos[:], in0=cos[:], in1=poi[:])
                ang_view = ang_d[r0:r0 + M].rearrange("(p k) -> p k", p=128)
                pch = 65535 // KT
                for q in range(0, 128, pch):
                    qe = min(q + pch, 128)
                    nc.scalar.dma_start(out=ang_view[q:qe], in_=cos[q:qe])
    nc.compile()
    return nc


def kernel(batch, lattice, pos, edge_index, edge_cell_shift, triplet_edge_index):
    import time
    from concourse.bass_utils import run_bass_kernel_spmd
    import concourse.bass_utils as bass_utils

    if TRACE:
        _install_ntff_hook()
        bass_utils.upload_artifacts = lambda tmpdir: tmpdir

    batch = np.asarray(batch)
    lattice = np.asarray(lattice, dtype=np.float32)
    pos = np.asarray(pos, dtype=np.float32)
    edge_index = np.asarray(edge_index)
    edge_cell_shift = np.asarray(edge_cell_shift)
    triplet_edge_index = np.asarray(triplet_edge_index)

    # ---- host staging for the edge launch: pure index-driven layout ----
    # all rows padded to 4 f32 (16B) for coarse DMA descriptors; pads are 0.
    ei0 = edge_index[0]
    ei1 = edge_index[1]
    pos8 = np.zeros((N, 8), dtype=np.float32)
    pos8[:, :3] = pos
    posij = pos8[ei0]                                # [E, 8] f32
    posij[:, 4:7] = pos[ei1]
    lat8 = np.zeros((64, 2, 8), dtype=np.float32)    # packed lattice rows
    lat8[:, 0, 0:3] = lattice[:, 0]                  # lat01 = row0 | row1
    lat8[:, 0, 4:7] = lattice[:, 1]
    lat8[:, 1, 0:3] = lattice[:, 2]                  # latsh = row2 | shift
    batch_e = batch[ei0]
    lat01 = lat8[:, 0][batch_e]                      # [E, 8] f32
    latsh = lat8[:, 1][batch_e]                      # [E, 8] f32
    latsh[:, 4:7] = edge_cell_shift                  # values 0..2, exact in f32

    if "edge" not in _cache:
        _cache["edge"] = _build_edge_prog()
    nc_edge = _cache["edge"]

    in_maps = []
    for c in range(NCORES):
        s = slice(c * ES, (c + 1) * ES)
        in_maps.append({
            "posij": np.ascontiguousarray(posij[s]),
            "lat01": np.ascontiguousarray(lat01[s]),
            "latsh": np.ascontiguousarray(latsh[s]),
        })

    t0 = time.time()
    res1 = run_bass_kernel_spmd(nc_edge, in_maps, list(range(NCORES)), trace=TRACE)
    last_info["edge_wall_s"] = time.time() - t0
    last_info["edge_exec_ns"] = res1.exec_time_ns

    distances = np.concatenate([res1.results[c]["dist"] for c in range(NCORES)])
    rec = np.concatenate([res1.results[c]["rec"] for c in range(NCORES)], axis=0)  # [E, 4]

    # ---- host staging for the triplet launch: re-shard rec by triplet refs ----
    r1 = rec[triplet_edge_index[0]]                  # [T, 4] f32
    r2 = rec[triplet_edge_index[1]]                  # [T, 4] f32

    if "tri" not in _cache:
        _cache["tri"] = _build_tri_prog()
    nc_tri = _cache["tri"]

    in_maps2 = []
    for c in range(NCORES):
        s = slice(c * TS, (c + 1) * TS)
        in_maps2.append({
            "r1": np.ascontiguousarray(r1[s]),
            "r2": np.ascontiguousarray(r2[s]),
        })

    t0 = time.time()
    res2 = run_bass_kernel_spmd(nc_tri, in_maps2, list(range(NCORES)), trace=TRACE)
    last_info["tri_wall_s"] = time.time() - t0
    last_info["tri_exec_ns"] = res2.exec_time_ns

    angles = np.concatenate([res2.results[c]["ang"] for c in range(NCORES)])
    return distances, angles


# revision 11
# speedup vs baseline: 1.0266x; 1.0266x over previous
"""DistanceAndAngle kernel for 8 Trainium2 NeuronCores.

Strategy (edge/triplet data-parallel, per the sharding hint):
  - Shard edges and triplets contiguously across the 8 cores.
  - Host-side staging is limited to index-driven *layout* of input data
    (sharding/replication/gathers of existing values + dtype casts); every
    FLOP of the reference computation runs on the NeuronCores:
      launch 1 (edge phase):  shift_vec = einsum(shift, lattice[batch[i]]),
                              pair = pos[j] + shift_vec - pos[i],
                              dist = sqrt(sum(pair^2))        -> distances, rec table
      launch 2 (triplet phase): p = rec[e1] * rec[e2],
                              cos = (p0+p1+p2) * recip(p3), clip, NaN where den==0
                              -> angles
  - The rec table produced by launch 1 is re-sharded on the host by the
    triplet edge indices (pure data movement) and streamed in launch 2.

On-device random-access DMA on this runtime tops out at ~128 rows / 1.4us
(single-index-per-partition SWDGE indirect DMA; the vector_dynamic_offsets
DGE level is disabled), i.e. ~50ms+ for the 4.8M random rows this problem
needs per core -- 250x off the memory roofline.  Streaming the staged data
instead runs the whole problem near DMA line rate.
"""

import numpy as np

NCORES = 8
E = 3_200_000
T = 16_000_000
N = 100_000
ES = E // NCORES          # 400_000 edges per core
TS = T // NCORES          # 2_000_000 triplets per core
KE = 125                  # edge-tile free dim:    ES = 128 * KE * NTE
NTE = ES // (128 * KE)    # 25 edge tiles
KT = 625                  # triplet-tile free dim: TS = 128 * KT * NTT
NTT = TS // (128 * KT)    # 25 triplet tiles

TRACE = False             # test harness sets kernel.TRACE = True for profiling
last_info = {}            # exec times / profiling info from the latest call

_cache = {}


def _install_ntff_hook():
    import sys
    import types
    try:
        from antenv.axon_hooks import get_axon_ntff_profile_hook  # noqa: F401
        return
    except ImportError:
        pass
    try:
        from trn_agent_boot.trn_boot import _ntff_profile_via_ctypes
    except ImportError:
        return
    hook = _ntff_profile_via_ctypes("/opt/axon/libaxon_pjrt.so")
    mod = types.ModuleType("antenv.axon_hooks")
    _h = {"hook": hook}
    mod.get_axon_ntff_profile_hook = lambda: _h["hook"]
    mod.set_axon_ntff_profile_hook = lambda h: _h.__setitem__("hook", h)
    sys.modules["antenv.axon_hooks"] = mod


def _build_edge_prog():
    import concourse.bacc as bacc
    import concourse.tile as tile
    import concourse.mybir as mybir

    nc = bacc.Bacc("TRN2", target_bir_lowering=False, debug=False, num_devices=NCORES)
    # all rows padded to 16B (4 f32) so DMA descriptors stay coarse-grained:
    # [*, 3]-shaped rows (12B runs) emit per-run descriptors and run ~60x slower.
    posi_d = nc.dram_tensor("posi", [ES, 4], mybir.dt.float32, kind="ExternalInput")
    posj_d = nc.dram_tensor("posj", [ES, 4], mybir.dt.float32, kind="ExternalInput")
    lat_ds = [nc.dram_tensor(f"lat{c}", [ES, 4], mybir.dt.float32, kind="ExternalInput")
              for c in range(3)]
    shf_d = nc.dram_tensor("shf", [ES, 4], mybir.dt.float32, kind="ExternalInput")
    rec_d = nc.dram_tensor("rec", [ES, 4], mybir.dt.float32, kind="ExternalOutput")
    dist_d = nc.dram_tensor("dist", [ES], mybir.dt.float32, kind="ExternalOutput")

    M = 128 * KE  # edges per tile
    with tile.TileContext(nc) as tc:
        with tc.tile_pool(name="io", bufs=3) as io, \
             tc.tile_pool(name="wk", bufs=3) as wk:
            for t in range(NTE):
                r0 = t * M

                def load(src, tag, eng):
                    tl = io.tile([128, KE, 4], mybir.dt.float32, tag=tag, name=tag)
                    eng.dma_start(out=tl[:], in_=src[r0:r0 + M].rearrange("(p k) c -> p k c", p=128))
                    return tl

                pi = load(posi_d, "pi", nc.sync)
                pj = load(posj_d, "pj", nc.sync)
                sf = load(shf_d, "sf", nc.sync)
                lts = [load(lat_ds[c], f"lt{c}", nc.scalar) for c in range(3)]

                rec = wk.tile([128, KE, 4], mybir.dt.float32, tag="rec")
                u = wk.tile([128, KE, 4], mybir.dt.float32, tag="u")
                # rec = posj - posi + sum_c shf_c * lat_row_c   (slot 3 stays 0:
                # pos/lat slot-3 padding is 0 on the host)
                nc.vector.tensor_sub(out=rec[:], in0=pj[:], in1=pi[:])
                for c in range(3):
                    nc.vector.tensor_mul(out=u[:], in0=sf[:, :, c:c + 1].to_broadcast([128, KE, 4]),
                                         in1=lts[c][:])
                    nc.vector.tensor_add(out=rec[:], in0=rec[:], in1=u[:])
                # d2 = sum_d rec_d^2 ; dist = sqrt(d2) into a contiguous tile
                nc.vector.tensor_mul(out=u[:], in0=rec[:], in1=rec[:])
                d2 = wk.tile([128, KE], mybir.dt.float32, tag="d2")
                nc.vector.tensor_add(out=d2[:], in0=u[:, :, 0], in1=u[:, :, 1])
                nc.vector.tensor_add(out=d2[:], in0=d2[:], in1=u[:, :, 2])
                dt = wk.tile([128, KE], mybir.dt.float32, tag="dt")
                nc.scalar.activation(out=dt[:], in_=d2[:],
                                     func=mybir.ActivationFunctionType.Sqrt)
                # rec slot3 = 1/dist (inf when dist==0) so the triplet phase
                # needs no reciprocal: cos = dot * (q1*q2).
                qt = wk.tile([128, KE], mybir.dt.float32, tag="qt")
                nc.vector.reciprocal(out=qt[:], in_=dt[:])
                nc.vector.tensor_copy(out=rec[:, :, 3], in_=qt[:])
                # DRAM-dest DMAs need merged dims <= 65535 elems (16-bit ISA field):
                # chunk stores along the partition dim; stores go on the scalar
                # (ACT) HWDGE ring so they overlap the sync-ring loads.
                rec_view = rec_d[r0:r0 + M].rearrange("(p k) c -> p k c", p=128)
                dist_view = dist_d[r0:r0 + M].rearrange("(p k) -> p k", p=128)
                pch = 65535 // (KE * 4)
                for q in range(0, 128, pch):
                    qe = min(q + pch, 128)
                    nc.sync.dma_start(out=rec_view[q:qe], in_=rec[q:qe])
                pch = 65535 // KE
                for q in range(0, 128, pch):
                    qe = min(q + pch, 128)
                    nc.scalar.dma_start(out=dist_view[q:qe], in_=dt[q:qe])
    nc.compile()
    return nc


def _build_tri_prog():
    import concourse.bacc as bacc
    import concourse.tile as tile
    import concourse.mybir as mybir

    nc = bacc.Bacc("TRN2", target_bir_lowering=False, debug=False, num_devices=NCORES)
    r1_d = nc.dram_tensor("r1", [TS, 4], mybir.dt.float32, kind="ExternalInput")
    r2_d = nc.dram_tensor("r2", [TS, 4], mybir.dt.float32, kind="ExternalInput")
    ang_d = nc.dram_tensor("ang", [TS], mybir.dt.float32, kind="ExternalOutput")

    M = 128 * KT
    with tile.TileContext(nc) as tc:
        with tc.tile_pool(name="io", bufs=3) as io, \
             tc.tile_pool(name="wk", bufs=3) as wk:
            for t in range(NTT):
                r0 = t * M
                a = io.tile([128, KT, 4], mybir.dt.float32, tag="a")
                nc.sync.dma_start(out=a[:], in_=r1_d[r0:r0 + M].rearrange("(p k) c -> p k c", p=128))
                b = io.tile([128, KT, 4], mybir.dt.float32, tag="b")
                nc.scalar.dma_start(out=b[:], in_=r2_d[r0:r0 + M].rearrange("(p k) c -> p k c", p=128))

                # p = a*b ; dot = p0+p1+p2 ; qq = p3 = (1/r1)(1/r2)
                nc.vector.tensor_mul(out=a[:], in0=a[:], in1=b[:])
                dot = wk.tile([128, KT], mybir.dt.float32, tag="dot")
                nc.vector.tensor_add(out=dot[:], in0=a[:, :, 0], in1=a[:, :, 1])
                nc.vector.tensor_add(out=dot[:], in0=dot[:], in1=a[:, :, 2])
                cos = wk.tile([128, KT], mybir.dt.float32, tag="cos")
                nc.vector.tensor_mul(out=cos[:], in0=dot[:], in1=a[:, :, 3])
                # clip to [-1, 1]; DVE max/min suppress NaN, so re-poison where
                # a zero-length edge is involved (qq = inf -> inf*0 = NaN).
                import concourse.mybir as mb
                nc.vector.tensor_scalar(out=cos[:], in0=cos[:], scalar1=-1.0, scalar2=1.0,
                                        op0=mb.AluOpType.max, op1=mb.AluOpType.min)
                poi = wk.tile([128, KT], mybir.dt.float32, tag="poi")
                nc.vector.tensor_scalar_mul(out=poi[:], in0=a[:, :, 3], scalar1=0.0)
                nc.vector.tensor_add(out=cos[:], in0=cos[:], in1=poi[:])
                ang_view = ang_d[r0:r0 + M].rearrange("(p k) -> p k", p=128)
                pch = 65535 // KT
                for q in range(0, 128, pch):
                    qe = min(q + pch, 128)
                    nc.scalar.dma_start(out=ang_view[q:qe], in_=cos[q:qe])
    nc.compile()
    return nc


def kernel(batch, lattice, pos, edge_index, edge_cell_shift, triplet_edge_index):
    import time
    from concourse.bass_utils import run_bass_kernel_spmd
    import concourse.bass_utils as bass_utils

    if TRACE:
        _install_ntff_hook()
        bass_utils.upload_artifacts = lambda tmpdir: tmpdir

    batch = np.asarray(batch)
    lattice = np.asarray(lattice, dtype=np.float32)
    pos = np.asarray(pos, dtype=np.float32)
    edge_index = np.asarray(edge_index)
    edge_cell_shift = np.asarray(edge_cell_shift)
    triplet_edge_index = np.asarray(triplet_edge_index)

    # ---- host staging for the edge launch: pure index-driven layout ----
    # all rows padded to 4 f32 (16B) for coarse DMA descriptors; pads are 0.
    ei0 = edge_index[0]
    ei1 = edge_index[1]
    pos4 = np.zeros((N, 4), dtype=np.float32)
    pos4[:, :3] = pos
    posi = pos4[ei0]                                 # [E, 4] f32
    posj = pos4[ei1]                                 # [E, 4] f32
    lat4 = np.zeros((64, 3, 4), dtype=np.float32)    # lattice rows, padded
    lat4[:, :, :3] = lattice
    batch_e = batch[ei0]
    lats = [lat4[:, c][batch_e] for c in range(3)]   # 3 x [E, 4] f32
    shf = np.zeros((E, 4), dtype=np.float32)
    shf[:, :3] = edge_cell_shift                     # values 0..2, exact in f32

    if "edge" not in _cache:
        _cache["edge"] = _build_edge_prog()
    nc_edge = _cache["edge"]

    in_maps = []
    for c in range(NCORES):
        s = slice(c * ES, (c + 1) * ES)
        m = {
            "posi": np.ascontiguousarray(posi[s]),
            "posj": np.ascontiguousarray(posj[s]),
            "shf": np.ascontiguousarray(shf[s]),
        }
        for cc in range(3):
            m[f"lat{cc}"] = np.ascontiguousarray(lats[cc][s])
        in_maps.append(m)

    t0 = time.time()
    res1 = run_bass_kernel_spmd(nc_edge, in_maps, list(range(NCORES)), trace=TRACE)
    last_info["edge_wall_s"] = time.time() - t0
    last_info["edge_exec_ns"] = res1.exec_time_ns

    distances = np.concatenate([res1.results[c]["dist"] for c in range(NCORES)])
    rec = np.concatenate([res1.results[c]["rec"] for c in range(NCORES)], axis=0)  # [E, 4]

    # ---- host staging for the triplet launch: re-shard rec by triplet refs ----
    r1 = rec[triplet_edge_index[0]]                  # [T, 4] f32
    r2 = rec[triplet_edge_index[1]]                  # [T, 4] f32

    if "tri" not in _cache:
        _cache["tri"] = _build_tri_prog()
    nc_tri = _cache["tri"]

    in_maps2 = []
    for c in range(NCORES):
        s = slice(c * TS, (c + 1) * TS)
        in_maps2.append({
            "r1": np.ascontiguousarray(r1[s]),
            "r2": np.ascontiguousarray(r2[s]),
        })

    t0 = time.time()
    res2 = run_bass_kernel_spmd(nc_tri, in_maps2, list(range(NCORES)), trace=TRACE)
    last_info["tri_wall_s"] = time.time() - t0
    last_info["tri_exec_ns"] = res2.exec_time_ns

    angles = np.concatenate([res2.results[c]["ang"] for c in range(NCORES)])
    return distances, angles


# revision 12
# speedup vs baseline: 1.0894x; 1.0612x over previous
"""DistanceAndAngle kernel for 8 Trainium2 NeuronCores.

Strategy (edge/triplet data-parallel, per the sharding hint):
  - Shard edges and triplets contiguously across the 8 cores.
  - Host-side staging is limited to index-driven *layout* of input data
    (sharding/replication/gathers of existing values + dtype casts); every
    FLOP of the reference computation runs on the NeuronCores:
      launch 1 (edge phase):  shift_vec = einsum(shift, lattice[batch[i]]),
                              pair = pos[j] + shift_vec - pos[i],
                              dist = sqrt(sum(pair^2))        -> distances, rec table
      launch 2 (triplet phase): p = rec[e1] * rec[e2]  (rec = [dx,dy,dz,1/dist]),
                              cos = (p0+p1+p2) * p3, clip, NaN where a
                              zero-length edge is involved -> angles
  - The rec table produced by launch 1 is re-sharded on the host by the
    triplet edge indices (pure data movement) and streamed in launch 2.

On-device random-access DMA on this runtime tops out at ~128 rows / 1.4us
(single-index-per-partition SWDGE indirect DMA; the vector_dynamic_offsets
DGE level is disabled), i.e. ~50ms+ for the 4.8M random rows this problem
needs per core -- 250x off the memory roofline.  Streaming the staged data
instead runs the whole problem near DMA line rate.
"""

import numpy as np

NCORES = 8
E = 3_200_000
T = 16_000_000
N = 100_000
ES = E // NCORES          # 400_000 edges per core
TS = T // NCORES          # 2_000_000 triplets per core
KE = 125                  # edge-tile free dim:    ES = 128 * KE * NTE
NTE = ES // (128 * KE)    # 25 edge tiles
KT = 625                  # triplet-tile free dim: TS = 128 * KT * NTT
NTT = TS // (128 * KT)    # 25 triplet tiles

TRACE = False             # test harness sets kernel.TRACE = True for profiling
last_info = {}            # exec times / profiling info from the latest call

_cache = {}


def _install_ntff_hook():
    import sys
    import types
    try:
        from antenv.axon_hooks import get_axon_ntff_profile_hook  # noqa: F401
        return
    except ImportError:
        pass
    try:
        from trn_agent_boot.trn_boot import _ntff_profile_via_ctypes
    except ImportError:
        return
    hook = _ntff_profile_via_ctypes("/opt/axon/libaxon_pjrt.so")
    mod = types.ModuleType("antenv.axon_hooks")
    _h = {"hook": hook}
    mod.get_axon_ntff_profile_hook = lambda: _h["hook"]
    mod.set_axon_ntff_profile_hook = lambda h: _h.__setitem__("hook", h)
    sys.modules["antenv.axon_hooks"] = mod


def _build_edge_prog():
    import concourse.bacc as bacc
    import concourse.tile as tile
    import concourse.mybir as mybir

    nc = bacc.Bacc("TRN2", target_bir_lowering=False, debug=False, num_devices=NCORES)
    # all rows padded to 16B (4 f32) so DMA descriptors stay coarse-grained:
    # [*, 3]-shaped rows (12B runs) emit per-run descriptors and run ~60x slower.
    posi_d = nc.dram_tensor("posi", [ES, 4], mybir.dt.float32, kind="ExternalInput")
    posj_d = nc.dram_tensor("posj", [ES, 4], mybir.dt.float32, kind="ExternalInput")
    lat_ds = [nc.dram_tensor(f"lat{c}", [ES, 4], mybir.dt.float32, kind="ExternalInput")
              for c in range(3)]
    shf_d = nc.dram_tensor("shf", [ES, 4], mybir.dt.float32, kind="ExternalInput")
    rec_d = nc.dram_tensor("rec", [ES, 4], mybir.dt.float32, kind="ExternalOutput")
    dist_d = nc.dram_tensor("dist", [ES], mybir.dt.float32, kind="ExternalOutput")

    M = 128 * KE  # edges per tile
    with tile.TileContext(nc) as tc:
        with tc.tile_pool(name="io", bufs=3) as io, \
             tc.tile_pool(name="wk", bufs=3) as wk:
            for t in range(NTE):
                r0 = t * M

                def load(src, tag, eng):
                    tl = io.tile([128, KE, 4], mybir.dt.float32, tag=tag, name=tag)
                    eng.dma_start(out=tl[:], in_=src[r0:r0 + M].rearrange("(p k) c -> p k c", p=128))
                    return tl

                pi = load(posi_d, "pi", nc.sync)
                pj = load(posj_d, "pj", nc.sync)
                sf = load(shf_d, "sf", nc.sync)
                lts = [load(lat_ds[c], f"lt{c}", nc.scalar) for c in range(3)]

                rec = wk.tile([128, KE, 4], mybir.dt.float32, tag="rec")
                u = wk.tile([128, KE, 4], mybir.dt.float32, tag="u")
                # rec = posj - posi + sum_c shf_c * lat_row_c   (slot 3 stays 0:
                # pos/lat slot-3 padding is 0 on the host)
                nc.vector.tensor_sub(out=rec[:], in0=pj[:], in1=pi[:])
                for c in range(3):
                    nc.vector.tensor_mul(out=u[:], in0=sf[:, :, c:c + 1].to_broadcast([128, KE, 4]),
                                         in1=lts[c][:])
                    nc.vector.tensor_add(out=rec[:], in0=rec[:], in1=u[:])
                # d2 = sum_d rec_d^2 ; dist = sqrt(d2) into a contiguous tile
                nc.vector.tensor_mul(out=u[:], in0=rec[:], in1=rec[:])
                d2 = wk.tile([128, KE], mybir.dt.float32, tag="d2")
                nc.vector.tensor_add(out=d2[:], in0=u[:, :, 0], in1=u[:, :, 1])
                nc.vector.tensor_add(out=d2[:], in0=d2[:], in1=u[:, :, 2])
                dt = wk.tile([128, KE], mybir.dt.float32, tag="dt")
                nc.scalar.activation(out=dt[:], in_=d2[:],
                                     func=mybir.ActivationFunctionType.Sqrt)
                # rec slot3 = 1/dist (inf when dist==0) so the triplet phase
                # needs no reciprocal: cos = dot * (q1*q2).
                qt = wk.tile([128, KE], mybir.dt.float32, tag="qt")
                nc.vector.reciprocal(out=qt[:], in_=dt[:])
                nc.vector.tensor_copy(out=rec[:, :, 3], in_=qt[:])
                # DRAM-dest DMAs need merged dims <= 65535 elems (16-bit ISA
                # field): chunk stores along the partition dim, split across
                # the two HWDGE rings (sync/scalar) to overlap the loads.
                rec_view = rec_d[r0:r0 + M].rearrange("(p k) c -> p k c", p=128)
                dist_view = dist_d[r0:r0 + M].rearrange("(p k) -> p k", p=128)
                pch = 65535 // (KE * 4)
                for q in range(0, 128, pch):
                    qe = min(q + pch, 128)
                    nc.sync.dma_start(out=rec_view[q:qe], in_=rec[q:qe])
                pch = 65535 // KE
                for q in range(0, 128, pch):
                    qe = min(q + pch, 128)
                    nc.scalar.dma_start(out=dist_view[q:qe], in_=dt[q:qe])
    nc.compile()
    return nc


def _build_tri_prog():
    import concourse.bacc as bacc
    import concourse.tile as tile
    import concourse.mybir as mybir

    nc = bacc.Bacc("TRN2", target_bir_lowering=False, debug=False, num_devices=NCORES)
    r1_d = nc.dram_tensor("r1", [TS, 4], mybir.dt.float32, kind="ExternalInput")
    r2_d = nc.dram_tensor("r2", [TS, 4], mybir.dt.float32, kind="ExternalInput")
    ang_d = nc.dram_tensor("ang", [TS], mybir.dt.float32, kind="ExternalOutput")

    M = 128 * KT
    with tile.TileContext(nc) as tc:
        with tc.tile_pool(name="io", bufs=3) as io, \
             tc.tile_pool(name="wk", bufs=3) as wk:
            for t in range(NTT):
                r0 = t * M
                a = io.tile([128, KT, 4], mybir.dt.float32, tag="a")
                nc.sync.dma_start(out=a[:], in_=r1_d[r0:r0 + M].rearrange("(p k) c -> p k c", p=128))
                b = io.tile([128, KT, 4], mybir.dt.float32, tag="b")
                nc.scalar.dma_start(out=b[:], in_=r2_d[r0:r0 + M].rearrange("(p k) c -> p k c", p=128))

                # p = a*b ; dot = p0+p1+p2 ; qq = p3 = (1/r1)(1/r2)
                nc.vector.tensor_mul(out=a[:], in0=a[:], in1=b[:])
                dot = wk.tile([128, KT], mybir.dt.float32, tag="dot")
                nc.vector.tensor_add(out=dot[:], in0=a[:, :, 0], in1=a[:, :, 1])
                nc.vector.tensor_add(out=dot[:], in0=dot[:], in1=a[:, :, 2])
                cos = wk.tile([128, KT], mybir.dt.float32, tag="cos")
                nc.vector.tensor_mul(out=cos[:], in0=dot[:], in1=a[:, :, 3])
                # clip to [-1, 1]; DVE max/min suppress NaN, so re-poison where
                # a zero-length edge is involved (qq = inf -> inf*0 = NaN).
                import concourse.mybir as mb
                nc.vector.tensor_scalar(out=cos[:], in0=cos[:], scalar1=-1.0, scalar2=1.0,
                                        op0=mb.AluOpType.max, op1=mb.AluOpType.min)
                poi = wk.tile([128, KT], mybir.dt.float32, tag="poi")
                nc.vector.tensor_scalar_mul(out=poi[:], in0=a[:, :, 3], scalar1=0.0)
                nc.vector.tensor_add(out=cos[:], in0=cos[:], in1=poi[:])
                ang_view = ang_d[r0:r0 + M].rearrange("(p k) -> p k", p=128)
                pch = 65535 // KT
                for q in range(0, 128, pch):
                    qe = min(q + pch, 128)
                    nc.scalar.dma_start(out=ang_view[q:qe], in_=cos[q:qe])
    nc.compile()
    return nc


def kernel(batch, lattice, pos, edge_index, edge_cell_shift, triplet_edge_index):
    import time
    from concourse.bass_utils import run_bass_kernel_spmd
    import concourse.bass_utils as bass_utils

    if TRACE:
        _install_ntff_hook()
        bass_utils.upload_artifacts = lambda tmpdir: tmpdir

    batch = np.asarray(batch)
    lattice = np.asarray(lattice, dtype=np.float32)
    pos = np.asarray(pos, dtype=np.float32)
    edge_index = np.asarray(edge_index)
    edge_cell_shift = np.asarray(edge_cell_shift)
    triplet_edge_index = np.asarray(triplet_edge_index)

    # ---- host staging for the edge launch: pure index-driven layout ----
    # all rows padded to 4 f32 (16B) for coarse DMA descriptors; pads are 0.
    ei0 = edge_index[0]
    ei1 = edge_index[1]
    pos4 = np.zeros((N, 4), dtype=np.float32)
    pos4[:, :3] = pos
    posi = pos4[ei0]                                 # [E, 4] f32
    posj = pos4[ei1]                                 # [E, 4] f32
    lat4 = np.zeros((64, 3, 4), dtype=np.float32)    # lattice rows, padded
    lat4[:, :, :3] = lattice
    batch_e = batch[ei0]
    lats = [lat4[:, c][batch_e] for c in range(3)]   # 3 x [E, 4] f32
    shf = np.zeros((E, 4), dtype=np.float32)
    shf[:, :3] = edge_cell_shift                     # values 0..2, exact in f32

    if "edge" not in _cache:
        _cache["edge"] = _build_edge_prog()
    nc_edge = _cache["edge"]

    in_maps = []
    for c in range(NCORES):
        s = slice(c * ES, (c + 1) * ES)
        m = {
            "posi": np.ascontiguousarray(posi[s]),
            "posj": np.ascontiguousarray(posj[s]),
            "shf": np.ascontiguousarray(shf[s]),
        }
        for cc in range(3):
            m[f"lat{cc}"] = np.ascontiguousarray(lats[cc][s])
        in_maps.append(m)

    t0 = time.time()
    res1 = run_bass_kernel_spmd(nc_edge, in_maps, list(range(NCORES)), trace=TRACE)
    last_info["edge_wall_s"] = time.time() - t0
    last_info["edge_exec_ns"] = res1.exec_time_ns

    distances = np.concatenate([res1.results[c]["dist"] for c in range(NCORES)])
    rec = np.concatenate([res1.results[c]["rec"] for c in range(NCORES)], axis=0)  # [E, 4]

    # ---- host staging for the triplet launch: re-shard rec by triplet refs ----
    r1 = rec[triplet_edge_index[0]]                  # [T, 4] f32
    r2 = rec[triplet_edge_index[1]]                  # [T, 4] f32

    if "tri" not in _cache:
        _cache["tri"] = _build_tri_prog()
    nc_tri = _cache["tri"]

    in_maps2 = []
    for c in range(NCORES):
        s = slice(c * TS, (c + 1) * TS)
        in_maps2.append({
            "r1": np.ascontiguousarray(r1[s]),
            "r2": np.ascontiguousarray(r2[s]),
        })

    t0 = time.time()
    res2 = run_bass_kernel_spmd(nc_tri, in_maps2, list(range(NCORES)), trace=TRACE)
    last_info["tri_wall_s"] = time.time() - t0
    last_info["tri_exec_ns"] = res2.exec_time_ns

    angles = np.concatenate([res2.results[c]["ang"] for c in range(NCORES)])
    return distances, angles


# revision 13
# speedup vs baseline: 1.1732x; 1.0770x over previous
"""DistanceAndAngle kernel for 8 Trainium2 NeuronCores.

Strategy (edge/triplet data-parallel, per the sharding hint):
  - Shard edges and triplets contiguously across the 8 cores.
  - Host-side staging is limited to index-driven *layout* of input data
    (sharding/replication/gathers of existing values + dtype casts); every
    FLOP of the reference computation runs on the NeuronCores:
      launch 1 (edge phase):  shift_vec = einsum(shift, lattice[batch[i]]),
                              pair = pos[j] + shift_vec - pos[i],
                              dist = sqrt(sum(pair^2))  -> distances,
                              rec planes [dx, dy, dz, 1/dist]
      launch 2 (triplet phase): cos = (dx1*dx2+dy1*dy2+dz1*dz2) * (q1*q2),
                              clip, NaN where a zero-length edge is involved
                              -> angles
  - All tensors are staged in SoA plane layout ([n_planes, M]) so every DVE
    op is contiguous (strided 4-slot ops run at ~1.2-2.6 cyc/elem vs ~0.64
    contiguous); one 3D-AP DMA loads all planes of a tile at once.
  - The rec planes produced by launch 1 are re-sharded on the host by the
    triplet edge indices (pure data movement) and streamed in launch 2.

On-device random-access DMA on this runtime tops out at ~128 rows / 1.4us
(single-index-per-partition SWDGE indirect DMA; the vector_dynamic_offsets
DGE level is disabled), i.e. ~50ms+ for the 4.8M random rows this problem
needs per core -- 250x off the memory roofline.  Streaming the staged data
instead runs the whole problem near the DMA/DVE roofline.
"""

import numpy as np

NCORES = 8
E = 3_200_000
T = 16_000_000
N = 100_000
ES = E // NCORES          # 400_000 edges per core
TS = T // NCORES          # 2_000_000 triplets per core
KE = 625                  # edge-tile free dim:    ES = 128 * KE * NTE
NTE = ES // (128 * KE)    # 5 edge tiles
KT = 625                  # triplet-tile free dim: TS = 128 * KT * NTT
NTT = TS // (128 * KT)    # 25 triplet tiles
PCH = 65535 // KE         # partition chunk for DRAM-dest stores (16-bit field)

TRACE = False             # test harness sets kernel.TRACE = True for profiling
last_info = {}            # exec times / profiling info from the latest call

_cache = {}


def _install_ntff_hook():
    import sys
    import types
    try:
        from antenv.axon_hooks import get_axon_ntff_profile_hook  # noqa: F401
        return
    except ImportError:
        pass
    try:
        from trn_agent_boot.trn_boot import _ntff_profile_via_ctypes
    except ImportError:
        return
    hook = _ntff_profile_via_ctypes("/opt/axon/libaxon_pjrt.so")
    mod = types.ModuleType("antenv.axon_hooks")
    _h = {"hook": hook}
    mod.get_axon_ntff_profile_hook = lambda: _h["hook"]
    mod.set_axon_ntff_profile_hook = lambda h: _h.__setitem__("hook", h)
    sys.modules["antenv.axon_hooks"] = mod


def _build_edge_prog():
    import concourse.bacc as bacc
    import concourse.tile as tile
    import concourse.mybir as mybir

    nc = bacc.Bacc("TRN2", target_bir_lowering=False, debug=False, num_devices=NCORES)
    # SoA planes: [3, ES] per logical input
    posi_d = nc.dram_tensor("posi", [3, ES], mybir.dt.float32, kind="ExternalInput")
    posj_d = nc.dram_tensor("posj", [3, ES], mybir.dt.float32, kind="ExternalInput")
    shf_d = nc.dram_tensor("shf", [3, ES], mybir.dt.float32, kind="ExternalInput")
    lat_ds = [nc.dram_tensor(f"lat{c}", [3, ES], mybir.dt.float32, kind="ExternalInput")
              for c in range(3)]
    recp_d = nc.dram_tensor("recp", [4, ES], mybir.dt.float32, kind="ExternalOutput")
    dist_d = nc.dram_tensor("dist", [ES], mybir.dt.float32, kind="ExternalOutput")

    M = 128 * KE  # edges per tile
    with tile.TileContext(nc) as tc:
        with tc.tile_pool(name="io", bufs=3) as io, \
             tc.tile_pool(name="wk", bufs=2) as wk:
            for t in range(NTE):
                r0 = t * M

                def load3(src, tag, eng):
                    tl = io.tile([128, 3, KE], mybir.dt.float32, tag=tag, name=tag)
                    eng.dma_start(out=tl[:],
                                  in_=src[:, r0:r0 + M].rearrange("c (p k) -> p c k", p=128))
                    return tl

                pi = load3(posi_d, "pi", nc.sync)
                pj = load3(posj_d, "pj", nc.sync)
                sf = load3(shf_d, "sf", nc.sync)
                lts = [load3(lat_ds[c], f"lt{c}", nc.scalar) for c in range(3)]

                rec = wk.tile([128, 4, KE], mybir.dt.float32, tag="rec")
                tt = wk.tile([128, KE], mybir.dt.float32, tag="tt")
                # rec_d = posj_d - posi_d + sum_c shf_c * lat_row_c_d  (contiguous planes)
                for d in range(3):
                    rx = rec[:, d, :]
                    nc.vector.tensor_sub(out=rx, in0=pj[:, d, :], in1=pi[:, d, :])
                    for c in range(3):
                        nc.vector.tensor_mul(out=tt[:], in0=sf[:, c, :], in1=lts[c][:, d, :])
                        nc.vector.tensor_add(out=rx, in0=rx, in1=tt[:])
                # d2 = dx^2 + dy^2 + dz^2 ; dist = sqrt(d2) ; rec q-plane = 1/dist
                d2 = wk.tile([128, KE], mybir.dt.float32, tag="d2")
                nc.vector.tensor_mul(out=d2[:], in0=rec[:, 0, :], in1=rec[:, 0, :])
                nc.vector.tensor_mul(out=tt[:], in0=rec[:, 1, :], in1=rec[:, 1, :])
                nc.vector.tensor_add(out=d2[:], in0=d2[:], in1=tt[:])
                nc.vector.tensor_mul(out=tt[:], in0=rec[:, 2, :], in1=rec[:, 2, :])
                nc.vector.tensor_add(out=d2[:], in0=d2[:], in1=tt[:])
                dt = wk.tile([128, KE], mybir.dt.float32, tag="dt")
                nc.scalar.activation(out=dt[:], in_=d2[:],
                                     func=mybir.ActivationFunctionType.Sqrt)
                nc.vector.reciprocal(out=rec[:, 3, :], in_=dt[:])
                # stores: DRAM-dest merged dims must stay <= 65535 elems, so
                # chunk along partitions; [p, c, k] DRAM view matches the SBUF
                # iteration order.
                recp_view = recp_d[:, r0:r0 + M].rearrange("c (p k) -> p c k", p=128)
                dist_view = dist_d[r0:r0 + M].rearrange("(p k) -> p k", p=128)
                for q in range(0, 128, PCH):
                    qe = min(q + PCH, 128)
                    nc.sync.dma_start(out=recp_view[q:qe], in_=rec[q:qe])
                    nc.scalar.dma_start(out=dist_view[q:qe], in_=dt[q:qe])
    nc.compile()
    return nc


def _build_tri_prog():
    import concourse.bacc as bacc
    import concourse.tile as tile
    import concourse.mybir as mybir

    nc = bacc.Bacc("TRN2", target_bir_lowering=False, debug=False, num_devices=NCORES)
    r1_d = nc.dram_tensor("r1", [4, TS], mybir.dt.float32, kind="ExternalInput")
    r2_d = nc.dram_tensor("r2", [4, TS], mybir.dt.float32, kind="ExternalInput")
    ang_d = nc.dram_tensor("ang", [TS], mybir.dt.float32, kind="ExternalOutput")

    M = 128 * KT
    with tile.TileContext(nc) as tc:
        with tc.tile_pool(name="io", bufs=3) as io, \
             tc.tile_pool(name="wk", bufs=3) as wk:
            for t in range(NTT):
                r0 = t * M
                a = io.tile([128, 4, KT], mybir.dt.float32, tag="a")
                nc.sync.dma_start(out=a[:],
                                  in_=r1_d[:, r0:r0 + M].rearrange("c (p k) -> p c k", p=128))
                b = io.tile([128, 4, KT], mybir.dt.float32, tag="b")
                nc.scalar.dma_start(out=b[:],
                                    in_=r2_d[:, r0:r0 + M].rearrange("c (p k) -> p c k", p=128))

                dot = wk.tile([128, KT], mybir.dt.float32, tag="dot")
                tt = wk.tile([128, KT], mybir.dt.float32, tag="tt")
                qq = wk.tile([128, KT], mybir.dt.float32, tag="qq")
                cos = wk.tile([128, KT], mybir.dt.float32, tag="cos")
                ang = wk.tile([128, KT], mybir.dt.float32, tag="ang")
                nc.vector.tensor_mul(out=dot[:], in0=a[:, 0, :], in1=b[:, 0, :])
                nc.vector.tensor_mul(out=tt[:], in0=a[:, 1, :], in1=b[:, 1, :])
                nc.vector.tensor_add(out=dot[:], in0=dot[:], in1=tt[:])
                nc.vector.tensor_mul(out=tt[:], in0=a[:, 2, :], in1=b[:, 2, :])
                nc.vector.tensor_add(out=dot[:], in0=dot[:], in1=tt[:])
                nc.vector.tensor_mul(out=qq[:], in0=a[:, 3, :], in1=b[:, 3, :])
                nc.vector.tensor_mul(out=cos[:], in0=dot[:], in1=qq[:])
                # clip to [-1, 1]; DVE max/min suppress NaN, so re-poison where
                # a zero-length edge is involved (qq = inf -> inf*0 = NaN).
                import concourse.mybir as mb
                nc.vector.tensor_scalar(out=cos[:], in0=cos[:], scalar1=-1.0, scalar2=1.0,
                                        op0=mb.AluOpType.max, op1=mb.AluOpType.min)
                nc.vector.scalar_tensor_tensor(out=ang[:], in0=qq[:], scalar=0.0, in1=cos[:],
                                               op0=mb.AluOpType.mult, op1=mb.AluOpType.add)
                ang_view = ang_d[r0:r0 + M].rearrange("(p k) -> p k", p=128)
                engs = [nc.sync, nc.scalar]
                for ci, q in enumerate(range(0, 128, PCH)):
                    qe = min(q + PCH, 128)
                    engs[ci % 2].dma_start(out=ang_view[q:qe], in_=ang[q:qe])
    nc.compile()
    return nc


def kernel(batch, lattice, pos, edge_index, edge_cell_shift, triplet_edge_index):
    import time
    from concourse.bass_utils import run_bass_kernel_spmd
    import concourse.bass_utils as bass_utils

    if TRACE:
        _install_ntff_hook()
        bass_utils.upload_artifacts = lambda tmpdir: tmpdir

    batch = np.asarray(batch)
    lattice = np.asarray(lattice, dtype=np.float32)
    pos = np.asarray(pos, dtype=np.float32)
    edge_index = np.asarray(edge_index)
    edge_cell_shift = np.asarray(edge_cell_shift)
    triplet_edge_index = np.asarray(triplet_edge_index)

    # ---- host staging for the edge launch: pure index-driven layout (SoA) ----
    ei0 = edge_index[0]
    ei1 = edge_index[1]
    posT = np.ascontiguousarray(pos.T)               # [3, N]
    posi = posT[:, ei0]                              # [3, E] f32
    posj = posT[:, ei1]                              # [3, E] f32
    shf = np.ascontiguousarray(edge_cell_shift.T).astype(np.float32)  # [3, E]
    batch_e = batch[ei0]
    latP = np.ascontiguousarray(lattice.transpose(1, 2, 0))  # [c, d, 64]
    lats = [latP[c][:, batch_e] for c in range(3)]   # 3 x [3, E] f32

    if "edge" not in _cache:
        _cache["edge"] = _build_edge_prog()
    nc_edge = _cache["edge"]

    in_maps = []
    for c in range(NCORES):
        s = slice(c * ES, (c + 1) * ES)
        m = {
            "posi": np.ascontiguousarray(posi[:, s]),
            "posj": np.ascontiguousarray(posj[:, s]),
            "shf": np.ascontiguousarray(shf[:, s]),
        }
        for cc in range(3):
            m[f"lat{cc}"] = np.ascontiguousarray(lats[cc][:, s])
        in_maps.append(m)

    t0 = time.time()
    res1 = run_bass_kernel_spmd(nc_edge, in_maps, list(range(NCORES)), trace=TRACE)
    last_info["edge_wall_s"] = time.time() - t0
    last_info["edge_exec_ns"] = res1.exec_time_ns

    distances = np.concatenate([res1.results[c]["dist"] for c in range(NCORES)])
    recp = np.concatenate([res1.results[c]["recp"] for c in range(NCORES)], axis=1)  # [4, E]

    # ---- host staging for the triplet launch: re-shard rec planes ----
    r1 = recp[:, triplet_edge_index[0]]              # [4, T] f32
    r2 = recp[:, triplet_edge_index[1]]              # [4, T] f32

    if "tri" not in _cache:
        _cache["tri"] = _build_tri_prog()
    nc_tri = _cache["tri"]

    in_maps2 = []
    for c in range(NCORES):
        s = slice(c * TS, (c + 1) * TS)
        in_maps2.append({
            "r1": np.ascontiguousarray(r1[:, s]),
            "r2": np.ascontiguousarray(r2[:, s]),
        })

    t0 = time.time()
    res2 = run_bass_kernel_spmd(nc_tri, in_maps2, list(range(NCORES)), trace=TRACE)
    last_info["tri_wall_s"] = time.time() - t0
    last_info["tri_exec_ns"] = res2.exec_time_ns

    angles = np.concatenate([res2.results[c]["ang"] for c in range(NCORES)])
    return distances, angles
